# revision 15
# baseline (speedup 1.0000x reference)
"""Trainium2 Bass kernel for AttentionAggregator (GAT-style message passing).

Reference math (per batch row b, N=32 neighbors, D=256):
    h_self      = self @ W                          [B, D]
    from_neighs = neigh @ W                         [B, N, D]
    logits      = from_neighs @ wa + self @ wa      where wa = W @ a
    coefs       = softmax(leaky_relu(logits), n)    [B, N]
    out         = leaky_relu(h_self + sum_n coefs * from_neighs + bias)

Key reassociations (exact up to fp32 rounding):
    sum_n coefs * (neigh @ W) = (sum_n coefs * neigh) @ W
    (neigh @ W) @ a           = neigh @ (W @ a)
so the big [B*N, D] @ [D, D] matmul disappears; every neigh row is read
once from HBM, dotted with wa (DVE tensor_tensor_reduce), and fed to the
PE as the moving operand of a weighted-sum matmul whose stationary is a
block-diagonal matrix of softmax coefficients.  u = self + sum_n c*neigh
is then transposed on the PE and multiplied by W.

Data-parallel over 8 NeuronCores: batch is split 8 ways, weights are
replicated, no collectives.
"""

import os
import sys
import numpy as np

for _p in ("/opt/trn_rl_repo",):
    if _p not in sys.path:
        sys.path.insert(0, _p)

import concourse.bass as bass
import concourse.bacc as bacc
import concourse.tile as tile
from concourse import mybir
from concourse.alu_op_type import AluOpType
from concourse.bass_utils import run_bass_kernel_spmd
from contextlib import ExitStack

FP = mybir.dt.float32
AF = mybir.ActivationFunctionType
P = 128          # SBUF partitions
D = 256          # feature dim
NN = 32          # neighbors
CB = 64          # batch rows per chunk (chunk = CB*NN = 2048 neigh rows)
TPC = CB * NN // P  # 16 r-tiles of [128, 256] per chunk
ALPHA = 0.2      # leaky_relu slope


def build(nc, b_loc, n_cores):
    """Emit the per-core program. b_loc must be a multiple of 128."""
    assert b_loc % P == 0
    CH = b_loc // CB          # chunks per core
    TS = b_loc // P           # self tiles == output b-tiles

    self_d = nc.dram_tensor("self_vecs", [b_loc, D], FP, kind="ExternalInput").ap()
    neigh_d = nc.dram_tensor("neigh", [b_loc * NN, D], FP, kind="ExternalInput").ap()
    waB_d = nc.dram_tensor("waB", [P, D], FP, kind="ExternalInput").ap()
    W_d = nc.dram_tensor("W", [2, P, D], FP, kind="ExternalInput").ap()
    S4_d = nc.dram_tensor("S4", [4, P], FP, kind="ExternalInput").ap()
    obd_d = nc.dram_tensor("ones_bd", [P, 4], FP, kind="ExternalInput").ap()
    I32_d = nc.dram_tensor("I32", [32, 32], FP, kind="ExternalInput").ap()
    I128_d = nc.dram_tensor("I128", [P, P], FP, kind="ExternalInput").ap()
    ones1_d = nc.dram_tensor("ones1", [1, P], FP, kind="ExternalInput").ap()
    biasr_d = nc.dram_tensor("bias_row", [1, D], FP, kind="ExternalInput").ap()
    out_d = nc.dram_tensor("out", [b_loc, D], FP, kind="ExternalOutput").ap()
    sl_dram = nc.dram_tensor("sl_scratch", [b_loc], FP).ap()

    with tile.TileContext(nc) as tc, ExitStack() as ctx:
        const = ctx.enter_context(tc.tile_pool(name="const", bufs=1))
        selfp = ctx.enter_context(tc.tile_pool(name="selfp", bufs=3))
        nvp = ctx.enter_context(tc.tile_pool(name="nvp", bufs=4))
        scr = ctx.enter_context(tc.tile_pool(name="scr", bufs=2))
        lp = ctx.enter_context(tc.tile_pool(name="lp", bufs=3))
        dp = ctx.enter_context(tc.tile_pool(name="dp", bufs=2))
        up = ctx.enter_context(tc.tile_pool(name="up", bufs=2))
        utp = ctx.enter_context(tc.tile_pool(name="utp", bufs=2))
        outp = ctx.enter_context(tc.tile_pool(name="outp", bufs=2))
        sps = ctx.enter_context(tc.tile_pool(name="sps", bufs=2, space=bass.MemorySpace.PSUM))
        ups = ctx.enter_context(tc.tile_pool(name="ups", bufs=2, space=bass.MemorySpace.PSUM))
        tps = ctx.enter_context(tc.tile_pool(name="tps", bufs=2, space=bass.MemorySpace.PSUM))
        ops = ctx.enter_context(tc.tile_pool(name="ops", bufs=2, space=bass.MemorySpace.PSUM))

        # ---- constants ----
        waB = const.tile([P, D], FP)
        nc.sync.dma_start(waB[:], waB_d)
        Wt = const.tile([P, 2, D], FP)
        nc.sync.dma_start(Wt[:], W_d.rearrange("h p d -> p h d"))
        S4 = const.tile([4, P], FP)
        nc.sync.dma_start(S4[:], S4_d)
        obd = const.tile([P, 4], FP)
        nc.sync.dma_start(obd[:], obd_d)
        I32 = const.tile([32, 32], FP)
        nc.sync.dma_start(I32[:], I32_d)
        I128 = const.tile([P, P], FP)
        nc.sync.dma_start(I128[:], I128_d)
        ones1 = const.tile([1, P], FP)
        nc.sync.dma_start(ones1[:], ones1_d)
        biasr = const.tile([1, D], FP)
        nc.sync.dma_start(biasr[:], biasr_d)

        # ---- phase 0: self logits sl[b] = self[b, :] . wa ----
        sl_all = const.tile([P, TS], FP)
        for ts in range(TS):
            st = selfp.tile([P, D], FP, tag="selftile")
            nc.sync.dma_start(st[:], self_d[P * ts:P * (ts + 1), :])
            z = scr.tile([P, D], FP, tag="ttr_scratch")
            nc.vector.scalar_tensor_tensor(
                z[:], st[:], 1.0, waB[:],
                op0=AluOpType.mult, op1=AluOpType.mult,
                accum_out=sl_all[:, ts:ts + 1])
        # bounce sl through DRAM in natural b order; per-chunk loads pick up
        # the [4, 16] layout sl4c[w, t] = sl[64c + 4t + w]
        nc.gpsimd.dma_start(sl_dram.rearrange("(ch p) -> p ch", p=P), sl_all[:])

        # ---- main loop over chunks of 64 batch rows ----
        psum_u = None
        for c in range(CH):
            nv = nvp.tile([P, TPC, D], FP, tag="nv")
            nc.sync.dma_start(
                nv[:],
                neigh_d[CB * NN * c:CB * NN * (c + 1), :].rearrange("(t p) d -> p t d", p=P))
            # two [32, D] self tiles so their base partition matches I32's
            sf = [selfp.tile([32, D], FP, tag=f"selfchunk{gp}", name=f"sf{gp}")
                  for gp in range(2)]
            for gp in range(2):
                nc.sync.dma_start(
                    sf[gp][:], self_d[CB * c + 32 * gp:CB * c + 32 * (gp + 1), :])

            # neighbor logits: one fused mul+reduce per [128, 256] tile
            logits = lp.tile([P, TPC], FP, tag="logits")
            for t in range(TPC):
                z = scr.tile([P, D], FP, tag="ttr_scratch")
                nc.vector.scalar_tensor_tensor(
                    z[:], nv[:, t, :], 1.0, waB[:],
                    op0=AluOpType.mult, op1=AluOpType.mult,
                    accum_out=logits[:, t:t + 1])

            # small psum scratch: slB | d4 | rB
            sp = sps.tile([P, 64], FP, tag="smallpsum")
            slB = sp[:, 0:TPC]
            d4 = sp[0:4, 16:16 + TPC]
            rB = sp[:, 32:32 + TPC]
            # broadcast self logits to [128, 16] chunk layout
            sl4c = lp.tile([4, TPC], FP, tag="sl4c")
            nc.gpsimd.dma_start(
                sl4c[:], sl_dram[CB * c:CB * (c + 1)].rearrange("(t w) -> w t", w=4))
            nc.tensor.matmul(slB, S4[:], sl4c[:], start=True, stop=True)
            l2 = lp.tile([P, TPC], FP, tag="l2")
            nc.vector.tensor_add(l2[:], logits[:], slB)
            l3 = lp.tile([P, TPC], FP, tag="l3")
            nc.vector.scalar_tensor_tensor(
                l3[:], l2[:], ALPHA, l2[:],
                op0=AluOpType.mult, op1=AluOpType.max)
            E = lp.tile([P, TPC], FP, tag="E")
            nc.scalar.activation(E[:], l3[:], AF.Exp)
            # denominators: sum exp over the 32 neighbors (partition groups of 32)
            nc.tensor.matmul(d4, obd[:], E[:], start=True, stop=True)
            r4 = lp.tile([4, TPC], FP, tag="r4")
            nc.vector.reciprocal(r4[:], d4)
            nc.tensor.matmul(rB, S4[:], r4[:], start=True, stop=True)
            coefs = lp.tile([P, TPC], FP, tag="coefs")
            nc.vector.tensor_mul(coefs[:], E[:], rB)

            # block-diagonal coefficient stationaries:
            # Dst[32w+v, 256g + 36tau + w] = coefs[32w+v, 8g+tau]
            Dst = dp.tile([P, 2 * D], FP, tag="dstack")
            nc.gpsimd.memset(Dst[:], 0.0)
            dst_base = Dst[:]
            pstride = dst_base.ap[0][0]
            for w in range(4):
                dst_ap = bass.AP(
                    dst_base.tensor,
                    dst_base.offset + 32 * w * pstride + w,
                    [[pstride, 32], [D, 2], [36, 8]])
                src_ap = coefs[32 * w:32 * (w + 1), :].rearrange("p (g t) -> p g t", g=2)
                nc.gpsimd.tensor_copy(dst_ap, src_ap)

            # weighted-sum matmuls: psum_u[32k+j, :] = sum_n coef*neigh + self
            if c % 2 == 0:
                psum_u = ups.tile([P, D], FP, tag="psum_u")
            for gp in range(2):
                k = 2 * (c % 2) + gp
                for tau in range(8):
                    nc.tensor.matmul(
                        psum_u[32 * k:32 * (k + 1), :],
                        Dst[:, D * gp + 32 * tau:D * gp + 32 * tau + 32],
                        nv[:, 8 * gp + tau, :],
                        start=(tau == 0), stop=False,
                        tile_position=(0, 32 * k))
                nc.tensor.matmul(
                    psum_u[32 * k:32 * (k + 1), :],
                    I32[:], sf[gp][:],
                    start=False, stop=True,
                    tile_position=(0, 32 * k))

            if c % 2 == 1:
                # u for 128 batch rows complete: transpose, multiply by W
                u128 = up.tile([P, D], FP, tag="u128")
                nc.scalar.copy(u128[:], psum_u[:])
                psum_o = ops.tile([P, D], FP, tag="psum_o")
                for dh in range(2):
                    puT = tps.tile([P, P], FP, tag="psum_uT")
                    nc.tensor.transpose(puT[:], u128[:, P * dh:P * (dh + 1)], I128[:])
                    uT = utp.tile([P, P], FP, tag="uT")
                    nc.scalar.copy(uT[:], puT[:])
                    nc.tensor.matmul(psum_o[:], uT[:], Wt[:, dh, :], start=(dh == 0), stop=False)
                nc.tensor.matmul(psum_o[:], ones1[:], biasr[:], start=False, stop=True)
                osb = outp.tile([P, D], FP, tag="osb")
                nc.scalar.copy(osb[:], psum_o[:])
                ot = outp.tile([P, D], FP, tag="outtile")
                nc.vector.scalar_tensor_tensor(
                    ot[:], osb[:], ALPHA, osb[:],
                    op0=AluOpType.mult, op1=AluOpType.max)
                bt = c // 2
                nc.sync.dma_start(out_d[P * bt:P * (bt + 1), :], ot[:])


def make_const_inputs(feat_weights, attn_weights, bias):
    """Host-side derived constants, replicated to every core."""
    wa = (feat_weights.astype(np.float64) @ attn_weights[:, 0].astype(np.float64))
    wa = wa.astype(np.float32)                      # [D]
    waB = np.broadcast_to(wa, (P, D)).copy()        # wa on every partition
    W = feat_weights.reshape(2, P, D).copy()        # d-half tiles
    S4 = np.zeros((4, P), np.float32)               # S4[w, p] = (p//32 == w)
    for w in range(4):
        S4[w, 32 * w:32 * (w + 1)] = 1.0
    obd = S4.T.copy()                               # [P, 4] partition-group ones
    I32 = np.eye(32, dtype=np.float32)
    I128 = np.eye(P, dtype=np.float32)
    ones1 = np.ones((1, P), np.float32)
    bias_row = bias.reshape(1, D).astype(np.float32)
    return {"waB": waB, "W": W, "S4": S4, "ones_bd": obd, "I32": I32,
            "I128": I128, "ones1": ones1, "bias_row": bias_row}


def kernel(self_vecs, neigh_vecs, feat_weights, attn_weights, bias):
    self_vecs = np.asarray(self_vecs, dtype=np.float32)
    neigh_vecs = np.asarray(neigh_vecs, dtype=np.float32)
    feat_weights = np.asarray(feat_weights, dtype=np.float32)
    attn_weights = np.asarray(attn_weights, dtype=np.float32)
    bias = np.asarray(bias, dtype=np.float32)

    B = self_vecs.shape[0]
    n_cores = 8
    b_loc = B // n_cores
    consts = make_const_inputs(feat_weights, attn_weights, bias)

    nc = bacc.Bacc("TRN2", target_bir_lowering=False, debug=False,
                   enable_asserts=False, num_devices=n_cores)
    build(nc, b_loc, n_cores)
    nc.compile()

    in_maps = []
    for i in range(n_cores):
        m = dict(consts)
        m["self_vecs"] = self_vecs[i * b_loc:(i + 1) * b_loc]
        m["neigh"] = neigh_vecs[i * b_loc:(i + 1) * b_loc].reshape(b_loc * NN, D)
        in_maps.append(m)

    res = run_bass_kernel_spmd(nc, in_maps, list(range(n_cores)))
    out = np.concatenate([res.results[i]["out"] for i in range(n_cores)], axis=0)
    return out


if __name__ == "__main__":
    B, Din, Dout = 16384, 256, 256
    rng = np.random.default_rng(0)
    inputs = {
        "self_vecs": rng.standard_normal((B, Din), dtype=np.float32),
        "neigh_vecs": rng.standard_normal((B, NN, Din), dtype=np.float32),
        "feat_weights": rng.standard_normal((Din, Dout), dtype=np.float32) * 0.0625,
        "attn_weights": rng.standard_normal((Dout, 1), dtype=np.float32) * 0.088,
        "bias": np.zeros(Dout, np.float32),
    }
    out = kernel(**inputs)
    print(out.shape, out.dtype)
